# revision 50
# baseline (speedup 1.0000x reference)
"""Single-head causal attention (B=4, T=4096, E=1024, H=128) on 8 trn2 cores.

Sharding: core c -> (batch b = c//2, piece p = c%2). Within a batch the 32
query blocks of 128 rows are split even/odd between the two pieces so the
causal workload balances. The device program is identical on all cores
(SPMD); per-core differences are carried by the input data (gathered query
columns xqT + a 0/1 causal-boundary mask strip).

Device algorithm (per core, "transposed" layouts):
  warmup: dummy matmuls at t~0 ramp the PE p-state during the DMA wait
  QT = Wq @ xq^T           [H=128, 2048]
  KT = Wk @ x^T            [H=128, 4096]
  VT = Wv @ x^T  -> PE-transpose -> V blocks [128 tok, 128 h]
  per q-tile (512 queries): for kb:
    ST[kb] = KT_blk^T @ QT_tile   [128 k, 512 q]  (PSUM)
    PT = exp(scale * ST)          (ACT, PSUM->SBUF f16)
    diag blocks: PT *= tri mask   (DVE f16, multiplicative - keeps ACT free)
    OT += V_blk^T @ PT            (PSUM accum)
    acc += PT                     (DVE SBUF f32 - softmax denominator)
  The NEXT round's projection matmuls are interleaved into the attention
  k-block stream so the PE never idles while ACT computes exp (idle PE
  triggers DVFS down-clocking that halves matmul throughput).
  epilogue per 128-q block i:
    lc_i = acc_blk^T @ ones; O_i = (OT_i)^T * (1/lc_i); 128-row store DMA
"""

import numpy as np

B, T, E, H = 4, 4096, 1024, 128
P = 128
NB_E = E // P           # 8 contraction chunks
TQ = T // 2             # 2048 gathered queries per core
N_QT = TQ // 512        # 4 q-tiles per core
SCALE = float(H) ** -0.5
N_CORES = 8
F32 = np.float32


def _query_rows(p: int) -> np.ndarray:
    """Absolute row indices of the gathered queries for piece p (in order)."""
    blocks = [np.arange(256 * g + 128 * p, 256 * g + 128 * p + 128) for g in range(16)]
    return np.concatenate(blocks)


def _mask_strip(p: int) -> np.ndarray:
    """mask [128 kk, 8 j, 512 q] f16: 1 where key visible, 0 where masked."""
    kk = np.arange(1024)[:, None]           # 128*j + kk
    qq = np.arange(512)[None, :]
    i, r = qq // 128, qq % 128
    visible = kk <= 256 * i + 128 * p + r
    m = visible.astype(np.float16)          # [1024, 512]
    return np.ascontiguousarray(m.reshape(8, 128, 512).transpose(1, 0, 2))


def _emit(tc, aps):
    import concourse.bass as bass
    from concourse import mybir
    from concourse.masks import make_identity

    nc = tc.nc
    f32 = mybir.dt.float32
    f16 = mybir.dt.float16
    EXP = mybir.ActivationFunctionType.Exp

    xT, xqT, wq, wk, wv, maskT, out = aps

    from contextlib import ExitStack

    ctx = ExitStack()
    with ctx:
        # ---- pools ----
        consts = ctx.enter_context(tc.tile_pool(name="consts", bufs=1))
        xp4 = ctx.enter_context(tc.tile_pool(name="xp4", bufs=6))
        xp8 = ctx.enter_context(tc.tile_pool(name="xp8", bufs=9))
        vt_pool = ctx.enter_context(tc.tile_pool(name="vt", bufs=2))
        pt_pool = ctx.enter_context(tc.tile_pool(name="pt", bufs=4))
        acc_pool = ctx.enter_context(tc.tile_pool(name="acc", bufs=2))
        osb_pool = ctx.enter_context(tc.tile_pool(name="osb", bufs=4))
        on_pool = ctx.enter_context(tc.tile_pool(name="on", bufs=4))
        sm_pool = ctx.enter_context(tc.tile_pool(name="sm", bufs=4))
        tmp_pool = ctx.enter_context(tc.tile_pool(name="tmp", bufs=2))
        s_ps = ctx.enter_context(tc.tile_pool(name="sps", bufs=2, space="PSUM"))
        p_ps = ctx.enter_context(tc.tile_pool(name="pps", bufs=2, space="PSUM"))
        o_ps = ctx.enter_context(tc.tile_pool(name="ops", bufs=2, space="PSUM"))
        t_ps = ctx.enter_context(tc.tile_pool(name="tps", bufs=2, space="PSUM"))

        # ---- warmup: ramp the PE clock while DMAs are in flight ----
        wu = consts.tile([P, 512], f16)
        nc.gpsimd.memset(wu[:], 0.0)
        wu_ps = t_ps.tile([P, 512], f32, tag="tps", name="warm")
        warm_n = [0]

        def warm(n, stop=False):
            """Dep-free PE matmuls in one long-lived accumulation group:
            emitted before DMA-gated instructions they keep the in-order PE
            busy (and the DVFS clock up) through the fill window."""
            for j in range(n):
                nc.tensor.matmul(
                    wu_ps[:], lhsT=wu[:, :P], rhs=wu[:],
                    start=(warm_n[0] == 0), stop=(stop and j == n - 1),
                )
                warm_n[0] += 1

        warm(16)

        # ---- persistent SBUF tensors ----
        identity = consts.tile([P, P], f16)
        ones32 = consts.tile([P, 1], f32)
        wq_sb = consts.tile([P, NB_E, P], f16)
        wk_sb = consts.tile([P, NB_E, P], f16)
        wv_sb = consts.tile([P, NB_E, P], f16)
        mask_sb = consts.tile([P, 8, 512], f16)
        kt_all = consts.tile([P, T], f16)
        v_all = consts.tile([P, T // P, P], f16)
        qt_all = consts.tile([P, TQ], f16)

        nc.vector.memset(ones32[:], 1.0)
        make_identity(nc, identity[:])

        # chunk-major views: one dma_start loads all 8 E-chunks of a tile
        xq_cm = xqT.rearrange("(c p) q -> p c q", p=P)
        xk_cm = xT.rearrange("(c p) t -> p c t", p=P)

        class XTile:
            """x tile split into DMA parts; chunk(c) -> [128, 512] AP."""
            def __init__(self, parts, cpp):
                self.parts, self.cpp = parts, cpp

            def chunk(self, c):
                return self.parts[c // self.cpp][:, c % self.cpp, :]

        def load_tile(cm, t0, nparts, nm):
            pool = {1: xp8, 2: xp4}[nparts]
            cpp = NB_E // nparts
            parts = []
            for h in range(nparts):
                xt = pool.tile([P, cpp, 512], f16, tag=pool.name,
                               name=f"{nm}_{h}")
                nc.sync.dma_start(
                    xt[:], cm[:, cpp * h:cpp * (h + 1), t0:t0 + 512])
                parts.append(xt)
            return XTile(parts, cpp)

        # ---- round-0 DMAs in latency order; later rounds prefetch ----
        xq_t = [None] * N_QT
        xk_t = [None] * (2 * N_QT)
        xq_t[0] = load_tile(xq_cm, 0, 2, "xq0")
        nc.sync.dma_start(wq_sb[:], wq)
        xk_t[0] = load_tile(xk_cm, 0, 2, "xk0")
        nc.sync.dma_start(wk_sb[:], wk)
        xk_t[1] = load_tile(xk_cm, 512, 2, "xk1")
        nc.sync.dma_start(wv_sb[:], wv)
        nc.sync.dma_start(mask_sb[:], maskT)
        for tt in range(1, N_QT):
            xq_t[tt] = load_tile(xq_cm, tt * 512, 1, f"xq{tt}")
            xk_t[2 * tt] = load_tile(xk_cm, 2 * tt * 512, 1, f"xk{2*tt}")
            xk_t[2 * tt + 1] = load_tile(
                xk_cm, (2 * tt + 1) * 512, 1, f"xk{2*tt+1}")

        # ---- projection work-item machinery ----
        # Each round's projections are emitted as a list of small closures
        # (one PE op each); the attention loop interleaves them between
        # score/PV matmuls so the PE pipeline never drains.
        _uid = [0]

        def proj_items(w_sb, x_tile, dst_ap, vt_tok=None):
            """Items for one projection [128, 512]. If vt_tok is not None,
            the result is V: transpose blocks into v_all instead of copy."""
            state = {}
            _uid[0] += 1
            uid = _uid[0]

            def mk_mm(c):
                def it():
                    if c == 0:
                        state["ps"] = p_ps.tile([P, 512], f32, tag="pps",
                                                name=f"pps_{uid}")
                    nc.tensor.matmul(
                        state["ps"][:],
                        lhsT=w_sb[:, c, :],
                        rhs=x_tile.chunk(c),
                        start=(c == 0),
                        stop=(c == NB_E - 1),
                    )
                    if c == NB_E - 1 and vt_tok is None:
                        nc.scalar.copy(dst_ap, state["ps"][:])
                return it

            items = [mk_mm(c) for c in range(NB_E)]
            if vt_tok is not None:
                def cp():
                    vt = vt_pool.tile([P, 512], f16, tag="vt",
                                      name=f"vt_{uid}")
                    state["vt"] = vt
                    nc.scalar.copy(vt[:], state["ps"][:])
                items.append(cp)

                def mk_tr(u):
                    def it():
                        kb = vt_tok * 4 + u
                        tp = t_ps.tile([P, P], f16, tag="tps",
                                       name=f"vtr_{kb}")
                        nc.tensor.transpose(
                            tp[:], state["vt"][:, u * P:(u + 1) * P],
                            identity[:])
                        nc.vector.tensor_copy(v_all[:, kb, :], tp[:])
                    return it
                items += [mk_tr(u) for u in range(4)]
            return items

        def g_q(tt):
            return proj_items(wq_sb, xq_t[tt],
                              qt_all[:, tt * 512:(tt + 1) * 512])

        def g_k(tok):
            return proj_items(wk_sb, xk_t[tok],
                              kt_all[:, tok * 512:(tok + 1) * 512])

        def g_v(tok):
            return proj_items(wv_sb, xk_t[tok], None, vt_tok=tok)

        def g_kv(tok):
            return g_k(tok) + g_v(tok)

        def dl(items, deadline):
            return [(deadline, it) for it in items]

        # ---- round 0 prologue: Q0 + K/V tok0 run straight (DMA-paced);
        # warm fillers after each group cover the next tile's DMA wait ----
        for idx, it in enumerate(g_q(0) + g_kv(0)):
            it()
            if idx == 7:
                warm(5)
            elif idx == 15:
                # close the warm group before V0's transposes need t_ps
                warm(3, stop=True)

        # second warm batch (o_ps is idle until attention 0 allocates ot):
        # absorbs the xk1 DMA wait that would otherwise stall attention 0's
        # interleaved K1 projection items
        wu2_ps = o_ps.tile([P, 512], f32, tag="ops", name="warm2")
        for j in range(3):
            nc.tensor.matmul(wu2_ps[:], lhsT=wu[:, :P], rhs=wu[:],
                             start=(j == 0), stop=(j == 2))

        # Work schedule: attention tt interleaves (deadline slot, item):
        #  - its own second token tile's K/V (needed from kb = 8tt+4)
        #  - projections needed before round tt+1 starts
        #  - round 2's epilogue is deferred into round 3's late slots
        work_of = {
            0: dl(g_kv(1), 2) + dl(g_q(1) + g_kv(2), 7),
            1: dl(g_kv(3), 10) + dl(g_q(2) + g_kv(4), 15),
            2: dl(g_kv(5), 18) + dl(g_q(3), 23),
            3: dl(g_k(6), 21) + dl(g_v(6), 22) + dl(g_k(7), 25)
               + dl(g_v(7), 27),
        }

        def epilogue_items(tt, ot, acc):
            state = {}

            def mk(i):
                def it():
                    if i == 0:
                        state["on"] = on_pool.tile([P, 4, P], f32, tag="on",
                                                   name=f"on_{tt}")
                    on = state["on"]
                    lc = t_ps.tile([P, 1], f32, tag="tps",
                                   name=f"lc_{tt}_{i}")
                    nc.tensor.matmul(
                        lc[:],
                        lhsT=acc[:, i * P:(i + 1) * P],
                        rhs=ones32[:],
                        start=True,
                        stop=True,
                    )
                    rlc = sm_pool.tile([P, 1], f32, tag="rlc",
                                       name=f"rlc_{tt}_{i}")
                    nc.vector.reciprocal(rlc[:], lc[:])
                    o_sb = osb_pool.tile([P, P], f16, tag="osb",
                                         name=f"osb_{tt}_{i}")
                    nc.vector.tensor_copy(o_sb[:], ot[:, i * P:(i + 1) * P])
                    tp = t_ps.tile([P, P], f16, tag="tps",
                                   name=f"otp_{tt}_{i}")
                    nc.tensor.transpose(tp[:], o_sb[:], identity[:])
                    nc.vector.tensor_scalar_mul(on[:, i, :], tp[:], rlc[:])
                    r0 = tt * 512 + i * P
                    nc.sync.dma_start(out[r0:r0 + P, :], on[:, i, :])
                return it
            return [mk(i) for i in range(4)]

        def filler_items(n, nm):
            """Dep-free PE matmuls: slotted into late attention slots that
            would otherwise idle the PE (and trigger DVFS down-clocking)."""
            def mk(j):
                def it():
                    # p_ps is idle during round 3 (no projections left)
                    fp = p_ps.tile([P, P], f32, tag="pps",
                                   name=f"fill_{nm}_{j}")
                    nc.tensor.matmul(fp[:], lhsT=wu[:, :P], rhs=wu[:, :P],
                                     start=True, stop=True)
                return it
            return [mk(j) for j in range(n)]

        # ---- attention rounds, fused with interleaved projections ----
        ep2 = None
        for tt in range(N_QT):
            work = work_of[tt]
            if tt == 3 and ep2 is not None:
                # alternate round-2 epilogue blocks with PE fillers through
                # the late slots
                tail = []
                for ii, ep in enumerate(ep2):
                    tail.append(ep)
                    tail += filler_items(3, f"e{ii}")
                work = work + dl(tail, 31)
            qs = qt_all[:, tt * 512:(tt + 1) * 512]
            ot = o_ps.tile([P, 512], f32, tag="ops")
            acc = acc_pool.tile([P, 512], f32, tag="acc")
            nkb = 8 * tt + 8

            s_tiles = [None] * nkb

            def c0_of(kb):
                if kb < 8 * tt:
                    return 0
                j = kb - 8 * tt
                return P * max(0, -(-(128 * j - 255) // 256))

            def emit_scores(kb):
                c0 = c0_of(kb)
                s = s_ps.tile([P, 512], f32, tag="sps", name=f"s_{tt}_{kb}")
                nc.tensor.matmul(
                    s[:, c0:512],
                    lhsT=kt_all[:, kb * P:(kb + 1) * P],
                    rhs=qs[:, c0:512],
                    start=True,
                    stop=True,
                )
                s_tiles[kb] = s

            emit_scores(0)
            wi = 0  # work items emitted
            pend = None  # pending pt tile for pair-summed accumulation
            for kb in range(nkb):
                if kb + 1 < nkb:
                    emit_scores(kb + 1)
                # interleave a fair share of the projection work; deadline
                # items are forced out regardless of the even-spread quota
                while wi < len(work) and (
                        work[wi][0] <= kb
                        or wi * nkb < (kb + 1) * len(work)):
                    work[wi][1]()
                    wi += 1
                s = s_tiles[kb]
                c0 = c0_of(kb)
                pt = pt_pool.tile([P, 512], f16, tag="pt")
                nc.scalar.activation(pt[:, c0:512], s[:, c0:512], EXP,
                                     scale=SCALE)
                if kb >= 8 * tt:
                    # zero the masked part of the boundary 128-col block
                    j = kb - 8 * tt
                    nc.vector.tensor_mul(
                        pt[:, c0:c0 + P], pt[:, c0:c0 + P],
                        mask_sb[:, j, c0:c0 + P])
                nc.tensor.matmul(
                    ot[:, c0:512],
                    lhsT=v_all[:, kb, :],
                    rhs=pt[:, c0:512],
                    start=(kb == 0),
                    stop=(kb == nkb - 1),
                )
                # softmax-denominator accumulation on DVE; full-width
                # blocks are pair-summed in f16 first (2x DVE rate)
                if kb == 0:
                    nc.vector.tensor_copy(acc[:], pt[:])
                elif c0 == 0:
                    if pend is None:
                        pend = pt
                    else:
                        tmp = tmp_pool.tile([P, 512], f16, tag="tmp",
                                            name=f"tmp_{tt}_{kb}")
                        nc.vector.tensor_add(tmp[:], pend[:], pt[:])
                        nc.vector.tensor_add(acc[:], acc[:], tmp[:])
                        pend = None
                else:
                    if pend is not None:
                        nc.vector.tensor_add(acc[:], acc[:], pend[:])
                        pend = None
                    nc.vector.tensor_add(acc[:, c0:512], acc[:, c0:512],
                                         pt[:, c0:512])
            if pend is not None:
                nc.vector.tensor_add(acc[:], acc[:], pend[:])
                pend = None
            assert wi == len(work)

            # epilogue: per-128-block normalize + transpose + store pipeline.
            # Round 2's epilogue is deferred into round 3's late slots
            # (which otherwise starve the PE while ACT computes exp).
            if tt == 2:
                ep2 = epilogue_items(2, ot, acc)
            elif tt < 2:
                for it in epilogue_items(tt, ot, acc):
                    it()
            else:
                # final tile: phase-ordered epilogue to shorten the tail.
                # All reciprocals/copies run on DVE before the first
                # transpose; the last two store triggers use the idle
                # Scalar queue instead of queueing behind Sync's.
                on = on_pool.tile([P, 4, P], f32, tag="on", name="on_f")
                rlcs, osbs = [], []
                for i in range(4):
                    lc = t_ps.tile([P, 1], f32, tag="tps", name=f"lcf_{i}")
                    nc.tensor.matmul(
                        lc[:], lhsT=acc[:, i * P:(i + 1) * P],
                        rhs=ones32[:], start=True, stop=True)
                    rlc = sm_pool.tile([P, 1], f32, tag="rlc",
                                       name=f"rlcf_{i}")
                    nc.vector.reciprocal(rlc[:], lc[:])
                    rlcs.append(rlc)
                    o_sb = osb_pool.tile([P, P], f16, tag="osb",
                                         name=f"osbf_{i}")
                    nc.vector.tensor_copy(o_sb[:], ot[:, i * P:(i + 1) * P])
                    osbs.append(o_sb)
                for i in range(4):
                    tp = t_ps.tile([P, P], f16, tag="tps", name=f"otpf_{i}")
                    nc.tensor.transpose(tp[:], osbs[i][:], identity[:])
                    nc.vector.tensor_scalar_mul(on[:, i, :], tp[:], rlcs[i][:])
                    r0 = tt * 512 + i * P
                    eng = nc.scalar if i >= 2 else nc.sync
                    eng.dma_start(out[r0:r0 + P, :], on[:, i, :])


def build_program():
    import concourse.tile as tile
    from concourse import bacc, mybir

    f32 = mybir.dt.float32
    f16 = mybir.dt.float16
    nc = bacc.Bacc("TRN2", target_bir_lowering=False, debug=False,
                   num_devices=N_CORES)
    xT = nc.dram_tensor("xT", [E, T], f16, kind="ExternalInput").ap()
    xqT = nc.dram_tensor("xqT", [E, TQ], f16, kind="ExternalInput").ap()
    wq = nc.dram_tensor("wq", [P, NB_E, P], f16, kind="ExternalInput").ap()
    wk = nc.dram_tensor("wk", [P, NB_E, P], f16, kind="ExternalInput").ap()
    wv = nc.dram_tensor("wv", [P, NB_E, P], f16, kind="ExternalInput").ap()
    maskT = nc.dram_tensor("maskT", [P, 8, 512], f16, kind="ExternalInput").ap()
    out = nc.dram_tensor("out", [TQ, H], f32, kind="ExternalOutput").ap()

    with tile.TileContext(nc) as tc:
        _emit(tc, (xT, xqT, wq, wk, wv, maskT, out))
    nc.compile()
    return nc


def _weight_pch(W: np.ndarray) -> np.ndarray:
    """[H, E] f32 -> [128(p), 8(c), 128(h)] f16 (p-major contiguous)."""
    wt = np.asarray(W, dtype=F32).T.astype(np.float16)          # [E, H]
    return np.ascontiguousarray(wt.reshape(NB_E, P, H).transpose(1, 0, 2))


def make_in_maps(x, Wq, Wk, Wv):
    """Per-core input maps. x: [B,T,E] f32; W*: [H,E] f32."""
    x = np.asarray(x, dtype=F32)
    wq_t = _weight_pch(Wq)
    wk_t = _weight_pch(Wk)
    wv_t = _weight_pch(Wv)
    in_maps = []
    for c in range(N_CORES):
        b, p = c // 2, c % 2
        xb = x[b]                                              # [T, E]
        in_maps.append({
            "xT": np.ascontiguousarray(xb.T.astype(np.float16)),
            "xqT": np.ascontiguousarray(
                xb[_query_rows(p)].T.astype(np.float16)),
            "maskT": _mask_strip(p),
            "wq": wq_t,
            "wk": wk_t,
            "wv": wv_t,
        })
    return in_maps


def run(x, Wq, Wk, Wv, trace=False, trace_cores=None):
    """Returns (full_output [B,T,H] f32, BassKernelResults)."""
    from concourse.bass_utils import run_bass_kernel_spmd

    nc = build_program()
    in_maps = make_in_maps(x, Wq, Wk, Wv)
    res = run_bass_kernel_spmd(
        nc, in_maps, list(range(N_CORES)), trace=trace,
        trace_cores=trace_cores,
    )
    full = np.empty((B, T, H), dtype=F32)
    for c in range(N_CORES):
        b, p = c // 2, c % 2
        full[b, _query_rows(p), :] = res.results[c]["out"]
    return full, res


def kernel(x, Wq, Wk, Wv):
    full, _ = run(x, Wq, Wk, Wv, trace=False)
    return full


if __name__ == "__main__":
    # quick smoke: build program only
    nc = build_program()
    print("program built ok")


# revision 51
# speedup vs baseline: 1.0112x; 1.0112x over previous
"""Single-head causal attention (B=4, T=4096, E=1024, H=128) on 8 trn2 cores.

Sharding: core c -> (batch b = c//2, piece p = c%2). Within a batch the 32
query blocks of 128 rows are split even/odd between the two pieces so the
causal workload balances. The device program is identical on all cores
(SPMD); per-core differences are carried by the input data (gathered query
columns xqT + a 0/1 causal-boundary mask strip).

Device algorithm (per core, "transposed" layouts):
  warmup: dummy matmuls at t~0 ramp the PE p-state during the DMA wait
  QT = Wq @ xq^T           [H=128, 2048]
  KT = Wk @ x^T            [H=128, 4096]
  VT = Wv @ x^T  -> PE-transpose -> V blocks [128 tok, 128 h]
  per q-tile (512 queries): for kb:
    ST[kb] = KT_blk^T @ QT_tile   [128 k, 512 q]  (PSUM)
    PT = exp(scale * ST)          (ACT, PSUM->SBUF f16)
    diag blocks: PT *= tri mask   (DVE f16, multiplicative - keeps ACT free)
    OT += V_blk^T @ PT            (PSUM accum)
    acc += PT                     (DVE SBUF f32 - softmax denominator)
  The NEXT round's projection matmuls are interleaved into the attention
  k-block stream so the PE never idles while ACT computes exp (idle PE
  triggers DVFS down-clocking that halves matmul throughput).
  epilogue per 128-q block i:
    lc_i = acc_blk^T @ ones; O_i = (OT_i)^T * (1/lc_i); 128-row store DMA
"""

import numpy as np

B, T, E, H = 4, 4096, 1024, 128
P = 128
NB_E = E // P           # 8 contraction chunks
TQ = T // 2             # 2048 gathered queries per core
N_QT = TQ // 512        # 4 q-tiles per core
SCALE = float(H) ** -0.5
N_CORES = 8
F32 = np.float32


def _query_rows(p: int) -> np.ndarray:
    """Absolute row indices of the gathered queries for piece p (in order)."""
    blocks = [np.arange(256 * g + 128 * p, 256 * g + 128 * p + 128) for g in range(16)]
    return np.concatenate(blocks)


def _mask_strip(p: int) -> np.ndarray:
    """mask [128 kk, 8 j, 512 q] f16: 1 where key visible, 0 where masked."""
    kk = np.arange(1024)[:, None]           # 128*j + kk
    qq = np.arange(512)[None, :]
    i, r = qq // 128, qq % 128
    visible = kk <= 256 * i + 128 * p + r
    m = visible.astype(np.float16)          # [1024, 512]
    return np.ascontiguousarray(m.reshape(8, 128, 512).transpose(1, 0, 2))


def _emit(tc, aps):
    import concourse.bass as bass
    from concourse import mybir
    from concourse.masks import make_identity

    nc = tc.nc
    f32 = mybir.dt.float32
    f16 = mybir.dt.float16
    EXP = mybir.ActivationFunctionType.Exp

    xT, xqT, wq, wk, wv, maskT, out = aps

    from contextlib import ExitStack

    ctx = ExitStack()
    with ctx:
        # ---- pools ----
        consts = ctx.enter_context(tc.tile_pool(name="consts", bufs=1))
        xp4 = ctx.enter_context(tc.tile_pool(name="xp4", bufs=6))
        xp8 = ctx.enter_context(tc.tile_pool(name="xp8", bufs=9))
        vt_pool = ctx.enter_context(tc.tile_pool(name="vt", bufs=2))
        pt_pool = ctx.enter_context(tc.tile_pool(name="pt", bufs=4))
        acc_pool = ctx.enter_context(tc.tile_pool(name="acc", bufs=2))
        osb_pool = ctx.enter_context(tc.tile_pool(name="osb", bufs=4))
        on_pool = ctx.enter_context(tc.tile_pool(name="on", bufs=4))
        sm_pool = ctx.enter_context(tc.tile_pool(name="sm", bufs=4))
        tmp_pool = ctx.enter_context(tc.tile_pool(name="tmp", bufs=2))
        s_ps = ctx.enter_context(tc.tile_pool(name="sps", bufs=2, space="PSUM"))
        p_ps = ctx.enter_context(tc.tile_pool(name="pps", bufs=2, space="PSUM"))
        o_ps = ctx.enter_context(tc.tile_pool(name="ops", bufs=2, space="PSUM"))
        t_ps = ctx.enter_context(tc.tile_pool(name="tps", bufs=2, space="PSUM"))

        # ---- warmup: ramp the PE clock while DMAs are in flight ----
        wu = consts.tile([P, 512], f16)
        nc.gpsimd.memset(wu[:], 0.0)
        wu_ps = t_ps.tile([P, 512], f32, tag="tps", name="warm")
        warm_n = [0]

        def warm(n, stop=False):
            """Dep-free PE matmuls in one long-lived accumulation group:
            emitted before DMA-gated instructions they keep the in-order PE
            busy (and the DVFS clock up) through the fill window."""
            for j in range(n):
                nc.tensor.matmul(
                    wu_ps[:], lhsT=wu[:, :P], rhs=wu[:],
                    start=(warm_n[0] == 0), stop=(stop and j == n - 1),
                )
                warm_n[0] += 1

        warm(16)

        # ---- persistent SBUF tensors ----
        identity = consts.tile([P, P], f16)
        ones32 = consts.tile([P, 1], f32)
        wq_sb = consts.tile([P, NB_E, P], f16)
        wk_sb = consts.tile([P, NB_E, P], f16)
        wv_sb = consts.tile([P, NB_E, P], f16)
        mask_sb = consts.tile([P, 8, 512], f16)
        kt_all = consts.tile([P, T], f16)
        v_all = consts.tile([P, T // P, P], f16)
        qt_all = consts.tile([P, TQ], f16)

        nc.vector.memset(ones32[:], 1.0)
        make_identity(nc, identity[:])

        # chunk-major views: one dma_start loads all 8 E-chunks of a tile
        xq_cm = xqT.rearrange("(c p) q -> p c q", p=P)
        xk_cm = xT.rearrange("(c p) t -> p c t", p=P)

        class XTile:
            """x tile split into DMA parts; chunk(c) -> [128, 512] AP."""
            def __init__(self, parts, cpp):
                self.parts, self.cpp = parts, cpp

            def chunk(self, c):
                return self.parts[c // self.cpp][:, c % self.cpp, :]

        def load_tile(cm, t0, nparts, nm):
            pool = {1: xp8, 2: xp4}[nparts]
            cpp = NB_E // nparts
            parts = []
            for h in range(nparts):
                xt = pool.tile([P, cpp, 512], f16, tag=pool.name,
                               name=f"{nm}_{h}")
                nc.sync.dma_start(
                    xt[:], cm[:, cpp * h:cpp * (h + 1), t0:t0 + 512])
                parts.append(xt)
            return XTile(parts, cpp)

        # ---- round-0 DMAs in latency order; later rounds prefetch ----
        xq_t = [None] * N_QT
        xk_t = [None] * (2 * N_QT)
        xq_t[0] = load_tile(xq_cm, 0, 2, "xq0")
        nc.sync.dma_start(wq_sb[:], wq)
        xk_t[0] = load_tile(xk_cm, 0, 2, "xk0")
        nc.sync.dma_start(wk_sb[:], wk)
        xk_t[1] = load_tile(xk_cm, 512, 2, "xk1")
        nc.sync.dma_start(wv_sb[:], wv)
        nc.sync.dma_start(mask_sb[:], maskT)
        for tt in range(1, N_QT):
            xq_t[tt] = load_tile(xq_cm, tt * 512, 1, f"xq{tt}")
            xk_t[2 * tt] = load_tile(xk_cm, 2 * tt * 512, 1, f"xk{2*tt}")
            xk_t[2 * tt + 1] = load_tile(
                xk_cm, (2 * tt + 1) * 512, 1, f"xk{2*tt+1}")

        # ---- projection work-item machinery ----
        # Each round's projections are emitted as a list of small closures
        # (one PE op each); the attention loop interleaves them between
        # score/PV matmuls so the PE pipeline never drains.
        _uid = [0]

        def proj_items(w_sb, x_tile, dst_ap, vt_tok=None):
            """Items for one projection [128, 512]. If vt_tok is not None,
            the result is V: transpose blocks into v_all instead of copy."""
            state = {}
            _uid[0] += 1
            uid = _uid[0]

            def mk_mm(c):
                def it():
                    if c == 0:
                        state["ps"] = p_ps.tile([P, 512], f32, tag="pps",
                                                name=f"pps_{uid}")
                    nc.tensor.matmul(
                        state["ps"][:],
                        lhsT=w_sb[:, c, :],
                        rhs=x_tile.chunk(c),
                        start=(c == 0),
                        stop=(c == NB_E - 1),
                    )
                    if c == NB_E - 1 and vt_tok is None:
                        nc.scalar.copy(dst_ap, state["ps"][:])
                return it

            items = [mk_mm(c) for c in range(NB_E)]
            if vt_tok is not None:
                def cp():
                    vt = vt_pool.tile([P, 512], f16, tag="vt",
                                      name=f"vt_{uid}")
                    state["vt"] = vt
                    nc.scalar.copy(vt[:], state["ps"][:])
                items.append(cp)

                def mk_tr(u):
                    def it():
                        kb = vt_tok * 4 + u
                        tp = t_ps.tile([P, P], f16, tag="tps",
                                       name=f"vtr_{kb}")
                        nc.tensor.transpose(
                            tp[:], state["vt"][:, u * P:(u + 1) * P],
                            identity[:])
                        nc.vector.tensor_copy(v_all[:, kb, :], tp[:])
                    return it
                items += [mk_tr(u) for u in range(4)]
            return items

        def g_q(tt):
            return proj_items(wq_sb, xq_t[tt],
                              qt_all[:, tt * 512:(tt + 1) * 512])

        def g_k(tok):
            return proj_items(wk_sb, xk_t[tok],
                              kt_all[:, tok * 512:(tok + 1) * 512])

        def g_v(tok):
            return proj_items(wv_sb, xk_t[tok], None, vt_tok=tok)

        def g_kv(tok):
            return g_k(tok) + g_v(tok)

        def dl(items, deadline):
            return [(deadline, it) for it in items]

        # ---- round 0 prologue: Q0 + K/V tok0 run straight (DMA-paced);
        # warm fillers after each group cover the next tile's DMA wait ----
        for idx, it in enumerate(g_q(0) + g_kv(0)):
            it()
            if idx == 7:
                warm(5)
            elif idx == 15:
                # close the warm group before V0's transposes need t_ps
                warm(3, stop=True)


        # Work schedule: attention tt interleaves (deadline slot, item):
        #  - its own second token tile's K/V (needed from kb = 8tt+4)
        #  - projections needed before round tt+1 starts
        #  - round 2's epilogue is deferred into round 3's late slots
        work_of = {
            0: dl(g_kv(1), 2) + dl(g_q(1) + g_kv(2), 7),
            1: dl(g_kv(3), 10) + dl(g_q(2) + g_kv(4), 15),
            2: dl(g_kv(5), 18) + dl(g_q(3), 23),
            3: dl(g_k(6), 21) + dl(g_v(6), 22) + dl(g_k(7), 25)
               + dl(g_v(7), 27),
        }

        def epilogue_items(tt, ot, acc):
            state = {}

            def mk(i):
                def it():
                    if i == 0:
                        state["on"] = on_pool.tile([P, 4, P], f32, tag="on",
                                                   name=f"on_{tt}")
                    on = state["on"]
                    lc = t_ps.tile([P, 1], f32, tag="tps",
                                   name=f"lc_{tt}_{i}")
                    nc.tensor.matmul(
                        lc[:],
                        lhsT=acc[:, i * P:(i + 1) * P],
                        rhs=ones32[:],
                        start=True,
                        stop=True,
                    )
                    rlc = sm_pool.tile([P, 1], f32, tag="rlc",
                                       name=f"rlc_{tt}_{i}")
                    nc.vector.reciprocal(rlc[:], lc[:])
                    o_sb = osb_pool.tile([P, P], f16, tag="osb",
                                         name=f"osb_{tt}_{i}")
                    nc.vector.tensor_copy(o_sb[:], ot[:, i * P:(i + 1) * P])
                    tp = t_ps.tile([P, P], f16, tag="tps",
                                   name=f"otp_{tt}_{i}")
                    nc.tensor.transpose(tp[:], o_sb[:], identity[:])
                    nc.vector.tensor_scalar_mul(on[:, i, :], tp[:], rlc[:])
                    r0 = tt * 512 + i * P
                    nc.sync.dma_start(out[r0:r0 + P, :], on[:, i, :])
                return it
            return [mk(i) for i in range(4)]

        def filler_items(n, nm):
            """Dep-free PE matmuls: slotted into late attention slots that
            would otherwise idle the PE (and trigger DVFS down-clocking)."""
            def mk(j):
                def it():
                    # p_ps is idle during round 3 (no projections left)
                    fp = p_ps.tile([P, P], f32, tag="pps",
                                   name=f"fill_{nm}_{j}")
                    nc.tensor.matmul(fp[:], lhsT=wu[:, :P], rhs=wu[:, :P],
                                     start=True, stop=True)
                return it
            return [mk(j) for j in range(n)]

        # ---- attention rounds, fused with interleaved projections ----
        ep2 = None
        for tt in range(N_QT):
            work = work_of[tt]
            if tt == 3 and ep2 is not None:
                # alternate round-2 epilogue blocks with PE fillers through
                # the late slots
                tail = []
                for ii, ep in enumerate(ep2):
                    tail.append(ep)
                    tail += filler_items(3, f"e{ii}")
                work = work + dl(tail, 31)
            qs = qt_all[:, tt * 512:(tt + 1) * 512]
            ot = o_ps.tile([P, 512], f32, tag="ops")
            acc = acc_pool.tile([P, 512], f32, tag="acc")
            nkb = 8 * tt + 8

            s_tiles = [None] * nkb

            def c0_of(kb):
                if kb < 8 * tt:
                    return 0
                j = kb - 8 * tt
                return P * max(0, -(-(128 * j - 255) // 256))

            def emit_scores(kb):
                c0 = c0_of(kb)
                s = s_ps.tile([P, 512], f32, tag="sps", name=f"s_{tt}_{kb}")
                nc.tensor.matmul(
                    s[:, c0:512],
                    lhsT=kt_all[:, kb * P:(kb + 1) * P],
                    rhs=qs[:, c0:512],
                    start=True,
                    stop=True,
                )
                s_tiles[kb] = s

            emit_scores(0)
            wi = 0  # work items emitted
            pend = None  # pending pt tile for pair-summed accumulation
            for kb in range(nkb):
                if kb + 1 < nkb:
                    emit_scores(kb + 1)
                # interleave a fair share of the projection work; deadline
                # items are forced out regardless of the even-spread quota
                while wi < len(work) and (
                        work[wi][0] <= kb
                        or wi * nkb < (kb + 1) * len(work)):
                    work[wi][1]()
                    wi += 1
                s = s_tiles[kb]
                c0 = c0_of(kb)
                pt = pt_pool.tile([P, 512], f16, tag="pt")
                nc.scalar.activation(pt[:, c0:512], s[:, c0:512], EXP,
                                     scale=SCALE)
                if kb >= 8 * tt:
                    # zero the masked part of the boundary 128-col block
                    j = kb - 8 * tt
                    nc.vector.tensor_mul(
                        pt[:, c0:c0 + P], pt[:, c0:c0 + P],
                        mask_sb[:, j, c0:c0 + P])
                nc.tensor.matmul(
                    ot[:, c0:512],
                    lhsT=v_all[:, kb, :],
                    rhs=pt[:, c0:512],
                    start=(kb == 0),
                    stop=(kb == nkb - 1),
                )
                # softmax-denominator accumulation on DVE; full-width
                # blocks are pair-summed in f16 first (2x DVE rate)
                if kb == 0:
                    nc.vector.tensor_copy(acc[:], pt[:])
                elif c0 == 0:
                    if pend is None:
                        pend = pt
                    else:
                        tmp = tmp_pool.tile([P, 512], f16, tag="tmp",
                                            name=f"tmp_{tt}_{kb}")
                        nc.vector.tensor_add(tmp[:], pend[:], pt[:])
                        nc.vector.tensor_add(acc[:], acc[:], tmp[:])
                        pend = None
                else:
                    if pend is not None:
                        nc.vector.tensor_add(acc[:], acc[:], pend[:])
                        pend = None
                    nc.vector.tensor_add(acc[:, c0:512], acc[:, c0:512],
                                         pt[:, c0:512])
            if pend is not None:
                nc.vector.tensor_add(acc[:], acc[:], pend[:])
                pend = None
            assert wi == len(work)

            # epilogue: per-128-block normalize + transpose + store pipeline.
            # Round 2's epilogue is deferred into round 3's late slots
            # (which otherwise starve the PE while ACT computes exp).
            if tt == 2:
                ep2 = epilogue_items(2, ot, acc)
            elif tt < 2:
                for it in epilogue_items(tt, ot, acc):
                    it()
            else:
                # final tile: phase-ordered epilogue to shorten the tail.
                # All reciprocals/copies run on DVE before the first
                # transpose; the last two store triggers use the idle
                # Scalar queue instead of queueing behind Sync's.
                on = on_pool.tile([P, 4, P], f32, tag="on", name="on_f")
                rlcs, osbs = [], []
                for i in range(4):
                    lc = t_ps.tile([P, 1], f32, tag="tps", name=f"lcf_{i}")
                    nc.tensor.matmul(
                        lc[:], lhsT=acc[:, i * P:(i + 1) * P],
                        rhs=ones32[:], start=True, stop=True)
                    rlc = sm_pool.tile([P, 1], f32, tag="rlc",
                                       name=f"rlcf_{i}")
                    nc.vector.reciprocal(rlc[:], lc[:])
                    rlcs.append(rlc)
                    o_sb = osb_pool.tile([P, P], f16, tag="osb",
                                         name=f"osbf_{i}")
                    nc.vector.tensor_copy(o_sb[:], ot[:, i * P:(i + 1) * P])
                    osbs.append(o_sb)
                for i in range(4):
                    tp = t_ps.tile([P, P], f16, tag="tps", name=f"otpf_{i}")
                    nc.tensor.transpose(tp[:], osbs[i][:], identity[:])
                    nc.vector.tensor_scalar_mul(on[:, i, :], tp[:], rlcs[i][:])
                    r0 = tt * 512 + i * P
                    eng = nc.scalar if i >= 2 else nc.sync
                    eng.dma_start(out[r0:r0 + P, :], on[:, i, :])


def build_program():
    import concourse.tile as tile
    from concourse import bacc, mybir

    f32 = mybir.dt.float32
    f16 = mybir.dt.float16
    nc = bacc.Bacc("TRN2", target_bir_lowering=False, debug=False,
                   num_devices=N_CORES)
    xT = nc.dram_tensor("xT", [E, T], f16, kind="ExternalInput").ap()
    xqT = nc.dram_tensor("xqT", [E, TQ], f16, kind="ExternalInput").ap()
    wq = nc.dram_tensor("wq", [P, NB_E, P], f16, kind="ExternalInput").ap()
    wk = nc.dram_tensor("wk", [P, NB_E, P], f16, kind="ExternalInput").ap()
    wv = nc.dram_tensor("wv", [P, NB_E, P], f16, kind="ExternalInput").ap()
    maskT = nc.dram_tensor("maskT", [P, 8, 512], f16, kind="ExternalInput").ap()
    out = nc.dram_tensor("out", [TQ, H], f32, kind="ExternalOutput").ap()

    with tile.TileContext(nc) as tc:
        _emit(tc, (xT, xqT, wq, wk, wv, maskT, out))
    nc.compile()
    return nc


def _weight_pch(W: np.ndarray) -> np.ndarray:
    """[H, E] f32 -> [128(p), 8(c), 128(h)] f16 (p-major contiguous)."""
    wt = np.asarray(W, dtype=F32).T.astype(np.float16)          # [E, H]
    return np.ascontiguousarray(wt.reshape(NB_E, P, H).transpose(1, 0, 2))


def make_in_maps(x, Wq, Wk, Wv):
    """Per-core input maps. x: [B,T,E] f32; W*: [H,E] f32."""
    x = np.asarray(x, dtype=F32)
    wq_t = _weight_pch(Wq)
    wk_t = _weight_pch(Wk)
    wv_t = _weight_pch(Wv)
    in_maps = []
    for c in range(N_CORES):
        b, p = c // 2, c % 2
        xb = x[b]                                              # [T, E]
        in_maps.append({
            "xT": np.ascontiguousarray(xb.T.astype(np.float16)),
            "xqT": np.ascontiguousarray(
                xb[_query_rows(p)].T.astype(np.float16)),
            "maskT": _mask_strip(p),
            "wq": wq_t,
            "wk": wk_t,
            "wv": wv_t,
        })
    return in_maps


def run(x, Wq, Wk, Wv, trace=False, trace_cores=None):
    """Returns (full_output [B,T,H] f32, BassKernelResults)."""
    from concourse.bass_utils import run_bass_kernel_spmd

    nc = build_program()
    in_maps = make_in_maps(x, Wq, Wk, Wv)
    res = run_bass_kernel_spmd(
        nc, in_maps, list(range(N_CORES)), trace=trace,
        trace_cores=trace_cores,
    )
    full = np.empty((B, T, H), dtype=F32)
    for c in range(N_CORES):
        b, p = c // 2, c % 2
        full[b, _query_rows(p), :] = res.results[c]["out"]
    return full, res


def kernel(x, Wq, Wk, Wv):
    full, _ = run(x, Wq, Wk, Wv, trace=False)
    return full


if __name__ == "__main__":
    # quick smoke: build program only
    nc = build_program()
    print("program built ok")


# revision 54
# speedup vs baseline: 1.0211x; 1.0098x over previous
"""Single-head causal attention (B=4, T=4096, E=1024, H=128) on 8 trn2 cores.

Sharding: core c -> (batch b = c//2, piece p = c%2). Within a batch the 32
query blocks of 128 rows are split even/odd between the two pieces so the
causal workload balances. The device program is identical on all cores
(SPMD); per-core differences are carried by the input data (gathered query
columns xqT + a 0/1 causal-boundary mask strip).

Device algorithm (per core, "transposed" layouts):
  warmup: dummy matmuls at t~0 ramp the PE p-state during the DMA wait
  QT = Wq @ xq^T           [H=128, 2048]
  KT = Wk @ x^T            [H=128, 4096]
  VT = Wv @ x^T  -> PE-transpose -> V blocks [128 tok, 128 h]
  per q-tile (512 queries): for kb:
    ST[kb] = KT_blk^T @ QT_tile   [128 k, 512 q]  (PSUM)
    PT = exp(scale * ST)          (ACT, PSUM->SBUF f16)
    diag blocks: PT *= tri mask   (DVE f16, multiplicative - keeps ACT free)
    OT += V_blk^T @ PT            (PSUM accum)
    acc += PT                     (DVE SBUF f32 - softmax denominator)
  The NEXT round's projection matmuls are interleaved into the attention
  k-block stream so the PE never idles while ACT computes exp (idle PE
  triggers DVFS down-clocking that halves matmul throughput).
  epilogue per 128-q block i:
    lc_i = acc_blk^T @ ones; O_i = (OT_i)^T * (1/lc_i); 128-row store DMA
"""

import numpy as np

B, T, E, H = 4, 4096, 1024, 128
P = 128
NB_E = E // P           # 8 contraction chunks
TQ = T // 2             # 2048 gathered queries per core
N_QT = TQ // 512        # 4 q-tiles per core
SCALE = float(H) ** -0.5
N_CORES = 8
F32 = np.float32


def _query_rows(p: int) -> np.ndarray:
    """Absolute row indices of the gathered queries for piece p (in order)."""
    blocks = [np.arange(256 * g + 128 * p, 256 * g + 128 * p + 128) for g in range(16)]
    return np.concatenate(blocks)


def _mask_strip(p: int) -> np.ndarray:
    """mask [128 kk, 8 j, 512 q] f16: 1 where key visible, 0 where masked."""
    kk = np.arange(1024)[:, None]           # 128*j + kk
    qq = np.arange(512)[None, :]
    i, r = qq // 128, qq % 128
    visible = kk <= 256 * i + 128 * p + r
    m = visible.astype(np.float16)          # [1024, 512]
    return np.ascontiguousarray(m.reshape(8, 128, 512).transpose(1, 0, 2))


def _emit(tc, aps):
    import concourse.bass as bass
    from concourse import mybir
    from concourse.masks import make_identity

    nc = tc.nc
    f32 = mybir.dt.float32
    f16 = mybir.dt.float16
    EXP = mybir.ActivationFunctionType.Exp

    xT, xqT, wq, wk, wv, maskT, out = aps

    from contextlib import ExitStack

    ctx = ExitStack()
    with ctx:
        # ---- pools ----
        consts = ctx.enter_context(tc.tile_pool(name="consts", bufs=1))
        xp4 = ctx.enter_context(tc.tile_pool(name="xp4", bufs=6))
        xp8 = ctx.enter_context(tc.tile_pool(name="xp8", bufs=9))
        vt_pool = ctx.enter_context(tc.tile_pool(name="vt", bufs=2))
        pt_pool = ctx.enter_context(tc.tile_pool(name="pt", bufs=4))
        acc_pool = ctx.enter_context(tc.tile_pool(name="acc", bufs=2))
        osb_pool = ctx.enter_context(tc.tile_pool(name="osb", bufs=4))
        on_pool = ctx.enter_context(tc.tile_pool(name="on", bufs=4))
        sm_pool = ctx.enter_context(tc.tile_pool(name="sm", bufs=4))
        tmp_pool = ctx.enter_context(tc.tile_pool(name="tmp", bufs=2))
        s_ps = ctx.enter_context(tc.tile_pool(name="sps", bufs=2, space="PSUM"))
        p_ps = ctx.enter_context(tc.tile_pool(name="pps", bufs=2, space="PSUM"))
        o_ps = ctx.enter_context(tc.tile_pool(name="ops", bufs=2, space="PSUM"))
        t_ps = ctx.enter_context(tc.tile_pool(name="tps", bufs=2, space="PSUM"))

        # ---- warmup: ramp the PE clock while DMAs are in flight ----
        wu = consts.tile([P, 512], f16)
        nc.gpsimd.memset(wu[:], 0.0)
        wu_ps = t_ps.tile([P, 512], f32, tag="tps", name="warm")
        warm_n = [0]

        def warm(n, stop=False):
            """Dep-free PE matmuls in one long-lived accumulation group:
            emitted before DMA-gated instructions they keep the in-order PE
            busy (and the DVFS clock up) through the fill window."""
            for j in range(n):
                nc.tensor.matmul(
                    wu_ps[:], lhsT=wu[:, :P], rhs=wu[:],
                    start=(warm_n[0] == 0), stop=(stop and j == n - 1),
                )
                warm_n[0] += 1

        warm(16)

        # ---- persistent SBUF tensors ----
        identity = consts.tile([P, P], f16)
        ones32 = consts.tile([P, 1], f32)
        wq_sb = consts.tile([P, NB_E, P], f16)
        wk_sb = consts.tile([P, NB_E, P], f16)
        wv_sb = consts.tile([P, NB_E, P], f16)
        mask_sb = consts.tile([P, 8, 512], f16)
        kt_all = consts.tile([P, T], f16)
        v_all = consts.tile([P, T // P, P], f16)
        qt_all = consts.tile([P, TQ], f16)

        nc.vector.memset(ones32[:], 1.0)
        make_identity(nc, identity[:])

        # chunk-major views: one dma_start loads all 8 E-chunks of a tile
        xq_cm = xqT.rearrange("(c p) q -> p c q", p=P)
        xk_cm = xT.rearrange("(c p) t -> p c t", p=P)

        class XTile:
            """x tile split into DMA parts; chunk(c) -> [128, 512] AP."""
            def __init__(self, parts, cpp):
                self.parts, self.cpp = parts, cpp

            def chunk(self, c):
                return self.parts[c // self.cpp][:, c % self.cpp, :]

        def load_tile(cm, t0, nparts, nm):
            pool = {1: xp8, 2: xp4}[nparts]
            cpp = NB_E // nparts
            parts = []
            for h in range(nparts):
                xt = pool.tile([P, cpp, 512], f16, tag=pool.name,
                               name=f"{nm}_{h}")
                nc.sync.dma_start(
                    xt[:], cm[:, cpp * h:cpp * (h + 1), t0:t0 + 512])
                parts.append(xt)
            return XTile(parts, cpp)

        # ---- round-0 DMAs in latency order; later rounds prefetch ----
        xq_t = [None] * N_QT
        xk_t = [None] * (2 * N_QT)
        xq_t[0] = load_tile(xq_cm, 0, 2, "xq0")
        nc.sync.dma_start(wq_sb[:], wq)
        xk_t[0] = load_tile(xk_cm, 0, 2, "xk0")
        nc.sync.dma_start(wk_sb[:], wk)
        xk_t[1] = load_tile(xk_cm, 512, 2, "xk1")
        nc.sync.dma_start(wv_sb[:], wv)
        nc.sync.dma_start(mask_sb[:], maskT)
        for tt in range(1, N_QT):
            xq_t[tt] = load_tile(xq_cm, tt * 512, 1, f"xq{tt}")
            xk_t[2 * tt] = load_tile(xk_cm, 2 * tt * 512, 1, f"xk{2*tt}")
            xk_t[2 * tt + 1] = load_tile(
                xk_cm, (2 * tt + 1) * 512, 1, f"xk{2*tt+1}")

        # ---- projection work-item machinery ----
        # Each round's projections are emitted as a list of small closures
        # (one PE op each); the attention loop interleaves them between
        # score/PV matmuls so the PE pipeline never drains.
        _uid = [0]

        def proj_items(w_sb, x_tile, dst_ap, vt_tok=None):
            """Items for one projection [128, 512]. If vt_tok is not None,
            the result is V: transpose blocks into v_all instead of copy."""
            state = {}
            _uid[0] += 1
            uid = _uid[0]

            def mk_mm(c):
                def it():
                    if c == 0:
                        state["ps"] = p_ps.tile([P, 512], f32, tag="pps",
                                                name=f"pps_{uid}")
                    nc.tensor.matmul(
                        state["ps"][:],
                        lhsT=w_sb[:, c, :],
                        rhs=x_tile.chunk(c),
                        start=(c == 0),
                        stop=(c == NB_E - 1),
                    )
                    if c == NB_E - 1 and vt_tok is None:
                        nc.scalar.copy(dst_ap, state["ps"][:])
                return it

            items = [mk_mm(c) for c in range(NB_E)]
            if vt_tok is not None:
                def cp():
                    vt = vt_pool.tile([P, 512], f16, tag="vt",
                                      name=f"vt_{uid}")
                    state["vt"] = vt
                    nc.scalar.copy(vt[:], state["ps"][:])
                items.append(cp)

                def mk_tr(u):
                    def it():
                        kb = vt_tok * 4 + u
                        tp = t_ps.tile([P, P], f16, tag="tps",
                                       name=f"vtr_{kb}")
                        nc.tensor.transpose(
                            tp[:], state["vt"][:, u * P:(u + 1) * P],
                            identity[:])
                        nc.vector.tensor_copy(v_all[:, kb, :], tp[:])
                    return it
                items += [mk_tr(u) for u in range(4)]
            return items

        def g_q(tt):
            return proj_items(wq_sb, xq_t[tt],
                              qt_all[:, tt * 512:(tt + 1) * 512])

        def g_k(tok):
            return proj_items(wk_sb, xk_t[tok],
                              kt_all[:, tok * 512:(tok + 1) * 512])

        def g_v(tok):
            return proj_items(wv_sb, xk_t[tok], None, vt_tok=tok)

        def g_kv(tok):
            return g_k(tok) + g_v(tok)

        def dl(items, deadline):
            return [(deadline, it) for it in items]

        # ---- round 0 prologue: Q0 + K/V tok0 run straight (DMA-paced);
        # warm fillers after each group cover the next tile's DMA wait ----
        for idx, it in enumerate(g_q(0) + g_kv(0)):
            it()
            if idx == 7:
                warm(5)
            elif idx == 15:
                # close the warm group before V0's transposes need t_ps
                warm(3, stop=True)


        # Work schedule: attention tt interleaves (deadline slot, item):
        #  - its own second token tile's K/V (needed from kb = 8tt+4)
        #  - projections needed before round tt+1 starts
        #  - round 2's epilogue is deferred into round 3's late slots
        work_of = {
            0: dl(g_kv(1), 2) + dl(g_q(1) + g_kv(2), 7),
            1: dl(g_kv(3), 10) + dl(g_q(2) + g_kv(4), 15),
            2: dl(g_kv(5), 18) + dl(g_q(3), 23),
            3: dl(g_k(6), 21) + dl(g_v(6), 22) + dl(g_k(7), 25)
               + dl(g_v(7), 27),
        }

        def epilogue_items(tt, ot, acc):
            state = {}

            def mk(i):
                def it():
                    if i == 0:
                        state["on"] = on_pool.tile([P, 4, P], f32, tag="on",
                                                   name=f"on_{tt}")
                    on = state["on"]
                    lc = t_ps.tile([P, 1], f32, tag="tps",
                                   name=f"lc_{tt}_{i}")
                    nc.tensor.matmul(
                        lc[:],
                        lhsT=acc[:, i * P:(i + 1) * P],
                        rhs=ones32[:],
                        start=True,
                        stop=True,
                    )
                    rlc = sm_pool.tile([P, 1], f32, tag="rlc",
                                       name=f"rlc_{tt}_{i}")
                    nc.vector.reciprocal(rlc[:], lc[:])
                    o_sb = osb_pool.tile([P, P], f16, tag="osb",
                                         name=f"osb_{tt}_{i}")
                    nc.vector.tensor_copy(o_sb[:], ot[:, i * P:(i + 1) * P])
                    tp = t_ps.tile([P, P], f16, tag="tps",
                                   name=f"otp_{tt}_{i}")
                    nc.tensor.transpose(tp[:], o_sb[:], identity[:])
                    nc.vector.tensor_scalar_mul(on[:, i, :], tp[:], rlc[:])
                    r0 = tt * 512 + i * P
                    nc.sync.dma_start(out[r0:r0 + P, :], on[:, i, :])
                return it
            return [mk(i) for i in range(4)]

        def filler_items(n, nm):
            """Dep-free PE matmuls: slotted into late attention slots that
            would otherwise idle the PE (and trigger DVFS down-clocking)."""
            def mk(j):
                def it():
                    # p_ps is idle during round 3 (no projections left)
                    fp = p_ps.tile([P, P], f32, tag="pps",
                                   name=f"fill_{nm}_{j}")
                    nc.tensor.matmul(fp[:], lhsT=wu[:, :P], rhs=wu[:, :P],
                                     start=True, stop=True)
                return it
            return [mk(j) for j in range(n)]

        # ---- attention rounds, fused with interleaved projections ----
        ep2 = None
        for tt in range(N_QT):
            work = work_of[tt]
            if tt == 3 and ep2 is not None:
                # alternate round-2 epilogue blocks with PE fillers through
                # the late slots
                tail = []
                for ii, ep in enumerate(ep2):
                    tail.append(ep)
                    tail += filler_items(3, f"e{ii}")
                work = work + dl(tail, 31)
            qs = qt_all[:, tt * 512:(tt + 1) * 512]
            ot = o_ps.tile([P, 512], f32, tag="ops")
            acc = acc_pool.tile([P, 512], f32, tag="acc")
            nkb = 8 * tt + 8

            s_tiles = [None] * nkb

            def c0_of(kb):
                if kb < 8 * tt:
                    return 0
                j = kb - 8 * tt
                return P * max(0, -(-(128 * j - 255) // 256))

            def emit_scores(kb):
                c0 = c0_of(kb)
                s = s_ps.tile([P, 512], f32, tag="sps", name=f"s_{tt}_{kb}")
                nc.tensor.matmul(
                    s[:, c0:512],
                    lhsT=kt_all[:, kb * P:(kb + 1) * P],
                    rhs=qs[:, c0:512],
                    start=True,
                    stop=True,
                )
                s_tiles[kb] = s

            emit_scores(0)
            wi = 0  # work items emitted
            pend = None  # pending pt tile for pair-summed accumulation

            def drain_work(kb):
                # interleave a fair share of the projection work; deadline
                # items are forced out regardless of the even-spread quota
                nonlocal wi
                while wi < len(work) and (
                        work[wi][0] <= kb
                        or wi * nkb < (kb + 1) * len(work)):
                    work[wi][1]()
                    wi += 1

            for kb in range(nkb):
                if kb + 1 < nkb:
                    emit_scores(kb + 1)
                # round 0's work items stall on round-0 DMAs: emit them
                # after PV so a stalled item can't block the PV matmul
                if tt > 0:
                    drain_work(kb)
                s = s_tiles[kb]
                c0 = c0_of(kb)
                pt = pt_pool.tile([P, 512], f16, tag="pt")
                nc.scalar.activation(pt[:, c0:512], s[:, c0:512], EXP,
                                     scale=SCALE)
                if kb >= 8 * tt:
                    # zero the masked part of the boundary 128-col block
                    j = kb - 8 * tt
                    nc.vector.tensor_mul(
                        pt[:, c0:c0 + P], pt[:, c0:c0 + P],
                        mask_sb[:, j, c0:c0 + P])
                nc.tensor.matmul(
                    ot[:, c0:512],
                    lhsT=v_all[:, kb, :],
                    rhs=pt[:, c0:512],
                    start=(kb == 0),
                    stop=(kb == nkb - 1),
                )
                # softmax-denominator accumulation on DVE; full-width
                # blocks are pair-summed in f16 first (2x DVE rate)
                if kb == 0:
                    nc.vector.tensor_copy(acc[:], pt[:])
                elif c0 == 0:
                    if pend is None:
                        pend = pt
                    else:
                        tmp = tmp_pool.tile([P, 512], f16, tag="tmp",
                                            name=f"tmp_{tt}_{kb}")
                        nc.vector.tensor_add(tmp[:], pend[:], pt[:])
                        nc.vector.tensor_add(acc[:], acc[:], tmp[:])
                        pend = None
                else:
                    if pend is not None:
                        nc.vector.tensor_add(acc[:], acc[:], pend[:])
                        pend = None
                    nc.vector.tensor_add(acc[:, c0:512], acc[:, c0:512],
                                         pt[:, c0:512])
                if tt == 0:
                    drain_work(kb)
            if pend is not None:
                nc.vector.tensor_add(acc[:], acc[:], pend[:])
                pend = None
            assert wi == len(work)

            # epilogue: per-128-block normalize + transpose + store pipeline.
            # Round 2's epilogue is deferred into round 3's late slots
            # (which otherwise starve the PE while ACT computes exp).
            if tt == 2:
                ep2 = epilogue_items(2, ot, acc)
            elif tt < 2:
                for it in epilogue_items(tt, ot, acc):
                    it()
            else:
                # final tile: phase-ordered epilogue to shorten the tail.
                # All reciprocals/copies run on DVE before the first
                # transpose; the last two store triggers use the idle
                # Scalar queue instead of queueing behind Sync's.
                on = on_pool.tile([P, 4, P], f32, tag="on", name="on_f")
                rlcs, osbs = [], []
                for i in range(4):
                    lc = t_ps.tile([P, 1], f32, tag="tps", name=f"lcf_{i}")
                    nc.tensor.matmul(
                        lc[:], lhsT=acc[:, i * P:(i + 1) * P],
                        rhs=ones32[:], start=True, stop=True)
                    for f in filler_items(1, f"fe1_{i}"):
                        f()
                    rlc = sm_pool.tile([P, 1], f32, tag="rlc",
                                       name=f"rlcf_{i}")
                    nc.vector.reciprocal(rlc[:], lc[:])
                    rlcs.append(rlc)
                    o_sb = osb_pool.tile([P, P], f16, tag="osb",
                                         name=f"osbf_{i}")
                    nc.vector.tensor_copy(o_sb[:], ot[:, i * P:(i + 1) * P])
                    osbs.append(o_sb)
                for i in range(4):
                    tp = t_ps.tile([P, P], f16, tag="tps", name=f"otpf_{i}")
                    nc.tensor.transpose(tp[:], osbs[i][:], identity[:])
                    for f in filler_items(2, f"fe2_{i}"):
                        f()
                    nc.vector.tensor_scalar_mul(on[:, i, :], tp[:], rlcs[i][:])
                    r0 = tt * 512 + i * P
                    if i == 3:
                        # split the last store across two idle engine
                        # queues so the final transfer halves
                        nc.sync.dma_start(out[r0:r0 + 64, :], on[:64, i, :])
                        nc.scalar.dma_start(out[r0 + 64:r0 + P, :],
                                            on[64:, i, :])
                    else:
                        eng = nc.scalar if i >= 2 else nc.sync
                        eng.dma_start(out[r0:r0 + P, :], on[:, i, :])


def build_program():
    import concourse.tile as tile
    from concourse import bacc, mybir

    f32 = mybir.dt.float32
    f16 = mybir.dt.float16
    nc = bacc.Bacc("TRN2", target_bir_lowering=False, debug=False,
                   num_devices=N_CORES)
    xT = nc.dram_tensor("xT", [E, T], f16, kind="ExternalInput").ap()
    xqT = nc.dram_tensor("xqT", [E, TQ], f16, kind="ExternalInput").ap()
    wq = nc.dram_tensor("wq", [P, NB_E, P], f16, kind="ExternalInput").ap()
    wk = nc.dram_tensor("wk", [P, NB_E, P], f16, kind="ExternalInput").ap()
    wv = nc.dram_tensor("wv", [P, NB_E, P], f16, kind="ExternalInput").ap()
    maskT = nc.dram_tensor("maskT", [P, 8, 512], f16, kind="ExternalInput").ap()
    out = nc.dram_tensor("out", [TQ, H], f32, kind="ExternalOutput").ap()

    with tile.TileContext(nc) as tc:
        _emit(tc, (xT, xqT, wq, wk, wv, maskT, out))
    nc.compile()
    return nc


def _weight_pch(W: np.ndarray) -> np.ndarray:
    """[H, E] f32 -> [128(p), 8(c), 128(h)] f16 (p-major contiguous)."""
    wt = np.asarray(W, dtype=F32).T.astype(np.float16)          # [E, H]
    return np.ascontiguousarray(wt.reshape(NB_E, P, H).transpose(1, 0, 2))


def make_in_maps(x, Wq, Wk, Wv):
    """Per-core input maps. x: [B,T,E] f32; W*: [H,E] f32."""
    x = np.asarray(x, dtype=F32)
    wq_t = _weight_pch(Wq)
    wk_t = _weight_pch(Wk)
    wv_t = _weight_pch(Wv)
    in_maps = []
    for c in range(N_CORES):
        b, p = c // 2, c % 2
        xb = x[b]                                              # [T, E]
        in_maps.append({
            "xT": np.ascontiguousarray(xb.T.astype(np.float16)),
            "xqT": np.ascontiguousarray(
                xb[_query_rows(p)].T.astype(np.float16)),
            "maskT": _mask_strip(p),
            "wq": wq_t,
            "wk": wk_t,
            "wv": wv_t,
        })
    return in_maps


def run(x, Wq, Wk, Wv, trace=False, trace_cores=None):
    """Returns (full_output [B,T,H] f32, BassKernelResults)."""
    from concourse.bass_utils import run_bass_kernel_spmd

    nc = build_program()
    in_maps = make_in_maps(x, Wq, Wk, Wv)
    res = run_bass_kernel_spmd(
        nc, in_maps, list(range(N_CORES)), trace=trace,
        trace_cores=trace_cores,
    )
    full = np.empty((B, T, H), dtype=F32)
    for c in range(N_CORES):
        b, p = c // 2, c % 2
        full[b, _query_rows(p), :] = res.results[c]["out"]
    return full, res


def kernel(x, Wq, Wk, Wv):
    full, _ = run(x, Wq, Wk, Wv, trace=False)
    return full


if __name__ == "__main__":
    # quick smoke: build program only
    nc = build_program()
    print("program built ok")
